# revision 4
# baseline (speedup 1.0000x reference)
"""Expected Calibration Error (ECE) kernel for Trainium2, 8 NeuronCores.

Problem: inputs [2e6, 128] f32 row-probabilities, targets [2e6] int64.
  conf_i = max_c inputs[i, c];  pred_i = argmax_c inputs[i, c]
  bin_i  = bucketize(conf_i, linspace(0, 1, 11), right=True) - 1
  ECE    = sum_b |corr_sum[b] - conf_sum[b]| / N

Strategy (data-parallel over rows, 250k rows per core):
  One custom fused DVE op per 128-row tile computes, per partition p
  (one row), streaming its 128 class probs v[c]:
      key[c] = round_to_mult_of_4(v[c] * 2^29) + (c == target_p)
      K[p]   = max(1, max_c key[c])
  The magic-number trick ((x + 2^25) - 2^25) rounds x < 2^24 to the nearest
  multiple of 4 exactly in fp32, and the +1 "target hit" bit is exact, so
      correct[p] = K - 4*rint(K/4)  in {0, 1}   (did the target attain the max)
      S4[p]      = K - correct[p]   = conf quantized to 2^-27, times 2^29.
  This is ONE DVE pass over the data (the memory-bound minimum).

  Keys are decoded in chunks on the (otherwise idle) GPSIMD engine into
  (S4, correct) pairs and cumulative >=-edge indicators G_b = [S4 >= e_b*2^29],
  then a tiny TensorE matmul per 128-row tile accumulates PSUM[2, 10]:
      out[0, b] = sum_i S4_i * G_b(i)      (scaled cumulative conf sums)
      out[1, b] = sum_i correct_i * G_b(i) (cumulative correct counts)
  All of that overlaps the DVE main loop.  Host finishes: per-bin values by
  differencing, |.| sum, / N.

Sharding: rows split evenly, 250,000 per core = 122 supertiles x 16 tiles
(p-major contiguous DMA) + 1 plain tile + 1 partial 16-row tile.
"""

import numpy as np

N = 2_000_000
C = 128
NCORES = 8
ROWS = N // NCORES            # 250_000
NST = 122                     # supertiles of 16 tiles (2048 rows each)
ST_ROWS = 128 * 16            # 2048
NT_MAIN = NST * 16            # 1952 tiles via supertiles
# tile 1952: 128 rows; tile 1953: 16 rows (partial)
NTG = NT_MAIN + 2             # 1954 key columns
PARTIAL_ROWS = ROWS - NST * ST_ROWS - 128  # 16

CHUNK = 256                   # key columns per decode/matmul chunk
NCHUNKS = (NTG + CHUNK - 1) // CHUNK       # 8 (last chunk = 162 cols)

SCALE_BITS = 29
SCALE = float(2 ** SCALE_BITS)
MAGIC = float(2 ** 25)
DEC_MAGIC = float(2 ** 23)

_EDGES_F32 = np.linspace(0.0, 1.0, 11).astype(np.float32)  # matches jnp.linspace
EDGES_SCALED = [float(_EDGES_F32[b]) * SCALE for b in range(10)]

_f32 = np.float32


def _ece_pack_ref(in0, in1, c0, c1, c2):
    P = in0.shape[0]
    x = in0.astype(np.float32).reshape(P, -1)
    n = x.shape[1]
    s = (x * _f32(c2)).astype(np.float32)
    r = ((s + _f32(c1)).astype(np.float32) - _f32(c1)).astype(np.float32)
    idx = np.arange(n, dtype=np.float32)[None, :]
    tgt = np.asarray(in1, np.float32).reshape(P, -1)[:, 0:1]
    key = (r + (idx == tgt).astype(np.float32)).astype(np.float32)
    acc = np.maximum(key.max(axis=1, keepdims=True), 1.0).astype(np.float32)
    return key, acc


def _register_op():
    from concourse.dve_ops import (
        DveOp,
        OPS,
        CUSTOM_DVE_SPECS,
        _SUB_OPCODE_FOR_NAME,
        _CUSTOM_DVE_ROW_BASE,
    )
    from concourse.dve_spec import (
        Spec,
        Src0,
        C1,
        C2,
        C3,
        One,
        eq,
        maxx,
        lower,
        Idx,
        _spill_c3_to_src1,
    )
    from concourse.dve_uop import DveOpSpec

    name = "ECE_PACK_ANT"
    if name in _SUB_OPCODE_FOR_NAME:
        return next(op for op in OPS if op.name == name)

    # target rides in1, read once at element 0 via the C3->Latch(Src1) spill
    body = ((Src0 * C2 + C1) - C1) + eq(Idx, C3)
    spec = Spec(
        body=_spill_c3_to_src1(body),
        accum=maxx,
        accum_init=One,
        reference=_ece_pack_ref,
    )

    row = _CUSTOM_DVE_ROW_BASE + len(OPS)
    assert row < 0x20
    _SUB_OPCODE_FOR_NAME[name] = row
    shas = {}
    for ver in ("v3", "v4"):
        try:
            uops = lower(spec, ver=ver)
            shas[ver] = DveOpSpec(
                name=name, opcode=row, uops=uops, rd1_en=True
            ).sha(ver)
        except Exception:
            pass
    op = DveOp(name, spec, subdim=False, uops_sha=shas)
    OPS.append(op)
    CUSTOM_DVE_SPECS[name] = spec
    return op


_NC_CACHE = None


def _build_bass():
    global _NC_CACHE
    if _NC_CACHE is not None:
        return _NC_CACHE

    import concourse.bacc as bacc
    import concourse.tile as tile
    from concourse import mybir

    ece_op = _register_op()

    nc = bacc.Bacc()
    f32 = mybir.dt.float32
    x = nc.dram_tensor("x", [ROWS, C], f32, kind="ExternalInput")
    tg = nc.dram_tensor("tg", [128, NTG], f32, kind="ExternalInput")
    out = nc.dram_tensor("out", [2, 10], f32, kind="ExternalOutput")

    with tile.TileContext(nc) as tc:
        with (
            tc.tile_pool(name="persist", bufs=1) as persist,
            tc.tile_pool(name="inbuf", bufs=3) as inbuf,
            tc.tile_pool(name="tailbuf", bufs=1) as tailbuf,
            tc.tile_pool(name="scratch", bufs=4) as scratch,
            tc.tile_pool(name="decbuf", bufs=2) as decbuf,
            tc.tile_pool(name="psum", bufs=1, space="PSUM") as psumpool,
        ):
            tg_buf = persist.tile([128, NTG], f32)
            nc.sync.dma_start(out=tg_buf[:], in_=tg[:])

            # one key tile per chunk so chunk decode only depends on its
            # own chunk's writers
            key_tiles = [
                persist.tile(
                    [128, min(CHUNK, NTG - c * CHUNK)], f32, name=f"key{c}", tag=f"key{c}"
                )
                for c in range(NCHUNKS)
            ]
            # partial-tile column: partitions 16.. are never written
            nc.gpsimd.memset(key_tiles[-1][:], 0.0)

            psum = psumpool.tile([2, 10], f32)

            x_ap = x[:]
            xr = x_ap[: NST * ST_ROWS, :].rearrange(
                "(s p k) c -> s p k c", s=NST, p=128, k=16
            )

            def emit_tile_op(in0_ap, j, nparts=128):
                c, l = divmod(j, CHUNK)
                dump = scratch.tile([128, C], f32, name="dump", tag="dump")
                nc.vector._custom_dve(
                    ece_op,
                    out=dump[:nparts, :],
                    in0=in0_ap,
                    in1=tg_buf[:nparts, j : j + 1],
                    s1=MAGIC,
                    imm2=SCALE,
                    accum_out=key_tiles[c][:nparts, l : l + 1],
                )

            def emit_chunk_epilogue(c):
                ncols = min(CHUNK, NTG - c * CHUNK)
                kt = key_tiles[c]
                cc = decbuf.tile([128, 2, CHUNK], f32, name="cc", tag="cc")
                g = decbuf.tile([128, 10, CHUNK], f32, name="g", tag="g")
                t1 = decbuf.tile([128, CHUNK], f32, name="t1", tag="t1")
                # t1 = rint(K/4) via (K*0.25 + 2^23) - 2^23, then *4
                nc.gpsimd.tensor_scalar(
                    out=t1[:, :ncols],
                    in0=kt[:, :ncols],
                    scalar1=0.25,
                    scalar2=DEC_MAGIC,
                    op0=mybir.AluOpType.mult,
                    op1=mybir.AluOpType.add,
                )
                nc.gpsimd.tensor_scalar(
                    out=t1[:, :ncols],
                    in0=t1[:, :ncols],
                    scalar1=DEC_MAGIC,
                    scalar2=4.0,
                    op0=mybir.AluOpType.subtract,
                    op1=mybir.AluOpType.mult,
                )
                nc.gpsimd.tensor_tensor(
                    out=cc[:, 1, :ncols],
                    in0=kt[:, :ncols],
                    in1=t1[:, :ncols],
                    op=mybir.AluOpType.subtract,
                )
                nc.gpsimd.tensor_tensor(
                    out=cc[:, 0, :ncols],
                    in0=kt[:, :ncols],
                    in1=cc[:, 1, :ncols],
                    op=mybir.AluOpType.subtract,
                )
                for b in range(10):
                    nc.gpsimd.tensor_scalar(
                        out=g[:, b, :ncols],
                        in0=cc[:, 0, :ncols],
                        scalar1=EDGES_SCALED[b],
                        scalar2=None,
                        op0=mybir.AluOpType.is_ge,
                    )
                for l in range(ncols):
                    j = c * CHUNK + l
                    nc.tensor.matmul(
                        psum[:],
                        lhsT=cc[:, :, l],
                        rhs=g[:, :, l],
                        start=(j == 0),
                        stop=(j == NTG - 1),
                    )

            for s in range(NST):
                xt = inbuf.tile([128, 16, C], f32, name="xt", tag="xt")
                nc.sync.dma_start(out=xt[:], in_=xr[s])
                for k in range(16):
                    emit_tile_op(xt[:, k, :], s * 16 + k)
                if (s + 1) % 16 == 0:
                    emit_chunk_epilogue((s + 1) // 16 - 1)

            # tail full tile (rows 249856:249984) -> column 1952
            xt2 = tailbuf.tile([128, C], f32)
            nc.sync.dma_start(
                out=xt2[:], in_=x_ap[NST * ST_ROWS : NST * ST_ROWS + 128, :]
            )
            emit_tile_op(xt2[:], NT_MAIN)

            # partial tile (16 rows, 249984:250000) -> column 1953
            xt3 = tailbuf.tile([PARTIAL_ROWS, C], f32)
            nc.sync.dma_start(out=xt3[:], in_=x_ap[NST * ST_ROWS + 128 :, :])
            emit_tile_op(xt3[:], NT_MAIN + 1, nparts=PARTIAL_ROWS)

            emit_chunk_epilogue(NCHUNKS - 1)

            res = persist.tile([2, 10], f32)
            nc.vector.tensor_copy(out=res[:], in_=psum[:])
            nc.sync.dma_start(out=out[:], in_=res[:])

    nc.finalize()
    _NC_CACHE = nc
    return nc


def _prep_targets(t_loc: np.ndarray) -> np.ndarray:
    """[ROWS] int targets -> [128, NTG] f32, laid out per tile."""
    s0 = t_loc.astype(np.float32)
    tg = np.zeros((128, NTG), dtype=np.float32)
    main = s0[: NST * ST_ROWS].reshape(NST, 128, 16)
    tg[:, :NT_MAIN] = main.transpose(1, 0, 2).reshape(128, NT_MAIN)
    tg[:, NT_MAIN] = s0[NST * ST_ROWS : NST * ST_ROWS + 128]
    tg[:PARTIAL_ROWS, NT_MAIN + 1] = s0[NST * ST_ROWS + 128 :]
    return tg


def _run(inputs: np.ndarray, targets: np.ndarray, trace: bool = False):
    from concourse.bass_utils import run_bass_kernel_spmd

    nc = _build_bass()

    inputs = np.ascontiguousarray(inputs, dtype=np.float32)
    targets = np.asarray(targets)

    in_maps = []
    for k in range(NCORES):
        lo = k * ROWS
        xs = inputs[lo : lo + ROWS]
        tgc = _prep_targets(targets[lo : lo + ROWS])
        in_maps.append({"x": xs, "tg": tgc})

    last_err = None
    for _attempt in range(3):
        try:
            r = run_bass_kernel_spmd(
                nc, in_maps, core_ids=list(range(NCORES)), trace=trace
            )
            break
        except Exception as e:  # transient NRT_EXEC_UNIT_UNRECOVERABLE on cold device
            last_err = e
    else:
        raise last_err
    return r


def _combine(results) -> np.ndarray:
    S = np.zeros((2, 10), dtype=np.float64)
    for r in results:
        S += r["out"].astype(np.float64)
    Sc = S[0] / SCALE
    Sk = S[1]
    conf_sum = Sc - np.append(Sc[1:], 0.0)
    corr_sum = Sk - np.append(Sk[1:], 0.0)
    ece = np.abs(corr_sum - conf_sum).sum() / N
    return np.asarray(ece, dtype=np.float32)


def kernel(inputs: np.ndarray, targets: np.ndarray) -> np.ndarray:
    r = _run(inputs, targets, trace=False)
    return _combine(r.results)


# revision 5
# speedup vs baseline: 1.4476x; 1.4476x over previous
"""Expected Calibration Error (ECE) kernel for Trainium2, 8 NeuronCores.

Problem: inputs [2e6, 128] f32 row-probabilities, targets [2e6] int64.
  conf_i = max_c inputs[i, c];  pred_i = argmax_c inputs[i, c]
  bin_i  = bucketize(conf_i, linspace(0, 1, 11), right=True) - 1
  ECE    = sum_b |corr_sum[b] - conf_sum[b]| / N

Strategy (data-parallel over rows, 250k rows per core):
  One custom fused DVE op per 128-row tile computes, per partition p
  (one row), streaming its 128 class probs v[c]:
      key[c] = round_to_mult_of_4(v[c] * 2^29) + (c == target_p)
      K[p]   = max(1, max_c key[c])
  The magic-number trick ((x + 2^25) - 2^25) rounds x < 2^24 to the nearest
  multiple of 4 exactly in fp32, and the +1 "target hit" bit is exact, so
      correct[p] = K - 4*rint(K/4)  in {0, 1}   (did the target attain the max)
      S4[p]      = K - correct[p]   = conf quantized to 2^-27, times 2^29.
  This is ONE DVE pass over the data (the memory-bound minimum).

  Keys are decoded in chunks on the (otherwise idle) GPSIMD engine into
  (S4, correct) pairs and cumulative >=-edge indicators G_b = [S4 >= e_b*2^29],
  then a tiny TensorE matmul per 128-row tile accumulates PSUM[2, 10]:
      out[0, b] = sum_i S4_i * G_b(i)      (scaled cumulative conf sums)
      out[1, b] = sum_i correct_i * G_b(i) (cumulative correct counts)
  All of that overlaps the DVE main loop.  Host finishes: per-bin values by
  differencing, |.| sum, / N.

Sharding: rows split evenly, 250,000 per core = 122 supertiles x 16 tiles
(p-major contiguous DMA) + 1 plain tile + 1 partial 16-row tile.
"""

import numpy as np

N = 2_000_000
C = 128
NCORES = 8
ROWS = N // NCORES            # 250_000
NST = 122                     # supertiles of 16 tiles (2048 rows each)
ST_ROWS = 128 * 16            # 2048
NT_MAIN = NST * 16            # 1952 tiles via supertiles
# tile 1952: 128 rows; tile 1953: 16 rows (partial)
NTG = NT_MAIN + 2             # 1954 key columns
PARTIAL_ROWS = ROWS - NST * ST_ROWS - 128  # 16

CHUNK = 256                   # key columns per decode/matmul chunk
NCHUNKS = (NTG + CHUNK - 1) // CHUNK       # 8 (last chunk = 162 cols)

SCALE_BITS = 29
SCALE = float(2 ** SCALE_BITS)
MAGIC = float(2 ** 25)
DEC_MAGIC = float(2 ** 23)

_EDGES_F32 = np.linspace(0.0, 1.0, 11).astype(np.float32)  # matches jnp.linspace
EDGES_SCALED = [float(_EDGES_F32[b]) * SCALE for b in range(10)]

_f32 = np.float32


def _ece_pack_ref(in0, in1, c0, c1, c2):
    P = in0.shape[0]
    x = in0.astype(np.float32).reshape(P, -1)
    n = x.shape[1]
    s = (x * _f32(c2)).astype(np.float32)
    r = ((s + _f32(c1)).astype(np.float32) - _f32(c1)).astype(np.float32)
    idx = np.arange(n, dtype=np.float32)[None, :]
    tgt = np.asarray(in1, np.float32).reshape(P, -1)[:, 0:1]
    key = (r + (idx == tgt).astype(np.float32)).astype(np.float32)
    acc = np.maximum(key.max(axis=1, keepdims=True), 1.0).astype(np.float32)
    return key, acc


def _register_op():
    from concourse.dve_ops import (
        DveOp,
        OPS,
        CUSTOM_DVE_SPECS,
        _SUB_OPCODE_FOR_NAME,
        _CUSTOM_DVE_ROW_BASE,
    )
    from concourse.dve_spec import (
        Spec,
        Src0,
        C1,
        C2,
        C3,
        One,
        eq,
        maxx,
        lower,
        Idx,
        _spill_c3_to_src1,
    )
    from concourse.dve_uop import DveOpSpec

    name = "ECE_PACK_ANT"
    if name in _SUB_OPCODE_FOR_NAME:
        return next(op for op in OPS if op.name == name)

    # target rides in1, read once at element 0 via the C3->Latch(Src1) spill
    body = ((Src0 * C2 + C1) - C1) + eq(Idx, C3)
    spec = Spec(
        body=_spill_c3_to_src1(body),
        accum=maxx,
        accum_init=One,
        reference=_ece_pack_ref,
    )

    row = _CUSTOM_DVE_ROW_BASE + len(OPS)
    assert row < 0x20
    _SUB_OPCODE_FOR_NAME[name] = row
    shas = {}
    for ver in ("v3", "v4"):
        try:
            uops = lower(spec, ver=ver)
            shas[ver] = DveOpSpec(
                name=name, opcode=row, uops=uops, rd1_en=True
            ).sha(ver)
        except Exception:
            pass
    op = DveOp(name, spec, subdim=False, uops_sha=shas)
    OPS.append(op)
    CUSTOM_DVE_SPECS[name] = spec
    return op


_NC_CACHE = None


def _build_bass():
    global _NC_CACHE
    if _NC_CACHE is not None:
        return _NC_CACHE

    import concourse.bacc as bacc
    import concourse.tile as tile
    from concourse import mybir

    ece_op = _register_op()

    nc = bacc.Bacc()
    f32 = mybir.dt.float32
    x = nc.dram_tensor("x", [ROWS, C], f32, kind="ExternalInput")
    tg = nc.dram_tensor("tg", [128, NTG], f32, kind="ExternalInput")
    out = nc.dram_tensor("out", [2, 10], f32, kind="ExternalOutput")

    with tile.TileContext(nc) as tc:
        with (
            tc.tile_pool(name="persist", bufs=1) as persist,
            tc.tile_pool(name="inbuf", bufs=3) as inbuf,
            tc.tile_pool(name="tailbuf", bufs=1) as tailbuf,
            tc.tile_pool(name="scratch", bufs=4) as scratch,
            tc.tile_pool(name="decbuf", bufs=2) as decbuf,
            tc.tile_pool(name="psum", bufs=1, space="PSUM") as psumpool,
        ):
            tg_buf = persist.tile([128, NTG], f32)
            nc.sync.dma_start(out=tg_buf[:], in_=tg[:])

            # one key tile per chunk so chunk decode only depends on its
            # own chunk's writers
            key_tiles = [
                persist.tile(
                    [128, min(CHUNK, NTG - c * CHUNK)], f32, name=f"key{c}", tag=f"key{c}"
                )
                for c in range(NCHUNKS)
            ]
            # partial-tile column: partitions 16.. are never written
            nc.gpsimd.memset(key_tiles[-1][:], 0.0)

            psum = psumpool.tile([2, 10], f32)

            x_ap = x[:]
            xr = x_ap[: NST * ST_ROWS, :].rearrange(
                "(s p k) c -> s p k c", s=NST, p=128, k=16
            )

            def emit_tile_op(in0_ap, j, nparts=128):
                c, l = divmod(j, CHUNK)
                dump = scratch.tile([128, C], f32, name="dump", tag="dump")
                nc.vector._custom_dve(
                    ece_op,
                    out=dump[:nparts, :],
                    in0=in0_ap,
                    in1=tg_buf[:nparts, j : j + 1],
                    s1=MAGIC,
                    imm2=SCALE,
                    accum_out=key_tiles[c][:nparts, l : l + 1],
                )

            def emit_chunk_epilogue(c):
                ncols = min(CHUNK, NTG - c * CHUNK)
                kt = key_tiles[c]
                cc = decbuf.tile([128, 2, CHUNK], f32, name="cc", tag="cc")
                g = decbuf.tile([128, 10, CHUNK], f32, name="g", tag="g")
                t1 = decbuf.tile([128, CHUNK], f32, name="t1", tag="t1")
                # t1 = rint(K/4) via (K*0.25 + 2^23) - 2^23, then *4
                nc.vector.tensor_scalar(
                    out=t1[:, :ncols],
                    in0=kt[:, :ncols],
                    scalar1=0.25,
                    scalar2=DEC_MAGIC,
                    op0=mybir.AluOpType.mult,
                    op1=mybir.AluOpType.add,
                )
                nc.vector.tensor_scalar(
                    out=t1[:, :ncols],
                    in0=t1[:, :ncols],
                    scalar1=DEC_MAGIC,
                    scalar2=4.0,
                    op0=mybir.AluOpType.subtract,
                    op1=mybir.AluOpType.mult,
                )
                nc.vector.tensor_tensor(
                    out=cc[:, 1, :ncols],
                    in0=kt[:, :ncols],
                    in1=t1[:, :ncols],
                    op=mybir.AluOpType.subtract,
                )
                nc.vector.tensor_tensor(
                    out=cc[:, 0, :ncols],
                    in0=kt[:, :ncols],
                    in1=cc[:, 1, :ncols],
                    op=mybir.AluOpType.subtract,
                )
                for b in range(10):
                    nc.vector.tensor_scalar(
                        out=g[:, b, :ncols],
                        in0=cc[:, 0, :ncols],
                        scalar1=EDGES_SCALED[b],
                        scalar2=None,
                        op0=mybir.AluOpType.is_ge,
                    )
                for l in range(ncols):
                    j = c * CHUNK + l
                    nc.tensor.matmul(
                        psum[:],
                        lhsT=cc[:, :, l],
                        rhs=g[:, :, l],
                        start=(j == 0),
                        stop=(j == NTG - 1),
                    )

            for s in range(NST):
                xt = inbuf.tile([128, 16, C], f32, name="xt", tag="xt")
                nc.sync.dma_start(out=xt[:], in_=xr[s])
                for k in range(16):
                    emit_tile_op(xt[:, k, :], s * 16 + k)
                if (s + 1) % 16 == 0:
                    emit_chunk_epilogue((s + 1) // 16 - 1)

            # tail full tile (rows 249856:249984) -> column 1952
            xt2 = tailbuf.tile([128, C], f32)
            nc.sync.dma_start(
                out=xt2[:], in_=x_ap[NST * ST_ROWS : NST * ST_ROWS + 128, :]
            )
            emit_tile_op(xt2[:], NT_MAIN)

            # partial tile (16 rows, 249984:250000) -> column 1953
            xt3 = tailbuf.tile([PARTIAL_ROWS, C], f32)
            nc.sync.dma_start(out=xt3[:], in_=x_ap[NST * ST_ROWS + 128 :, :])
            emit_tile_op(xt3[:], NT_MAIN + 1, nparts=PARTIAL_ROWS)

            emit_chunk_epilogue(NCHUNKS - 1)

            res = persist.tile([2, 10], f32)
            nc.vector.tensor_copy(out=res[:], in_=psum[:])
            nc.sync.dma_start(out=out[:], in_=res[:])

    nc.finalize()
    _NC_CACHE = nc
    return nc


def _prep_targets(t_loc: np.ndarray) -> np.ndarray:
    """[ROWS] int targets -> [128, NTG] f32, laid out per tile."""
    s0 = t_loc.astype(np.float32)
    tg = np.zeros((128, NTG), dtype=np.float32)
    main = s0[: NST * ST_ROWS].reshape(NST, 128, 16)
    tg[:, :NT_MAIN] = main.transpose(1, 0, 2).reshape(128, NT_MAIN)
    tg[:, NT_MAIN] = s0[NST * ST_ROWS : NST * ST_ROWS + 128]
    tg[:PARTIAL_ROWS, NT_MAIN + 1] = s0[NST * ST_ROWS + 128 :]
    return tg


def _run(inputs: np.ndarray, targets: np.ndarray, trace: bool = False):
    from concourse.bass_utils import run_bass_kernel_spmd

    nc = _build_bass()

    inputs = np.ascontiguousarray(inputs, dtype=np.float32)
    targets = np.asarray(targets)

    in_maps = []
    for k in range(NCORES):
        lo = k * ROWS
        xs = inputs[lo : lo + ROWS]
        tgc = _prep_targets(targets[lo : lo + ROWS])
        in_maps.append({"x": xs, "tg": tgc})

    last_err = None
    for _attempt in range(3):
        try:
            r = run_bass_kernel_spmd(
                nc, in_maps, core_ids=list(range(NCORES)), trace=trace
            )
            break
        except Exception as e:  # transient NRT_EXEC_UNIT_UNRECOVERABLE on cold device
            last_err = e
    else:
        raise last_err
    return r


def _combine(results) -> np.ndarray:
    S = np.zeros((2, 10), dtype=np.float64)
    for r in results:
        S += r["out"].astype(np.float64)
    Sc = S[0] / SCALE
    Sk = S[1]
    conf_sum = Sc - np.append(Sc[1:], 0.0)
    corr_sum = Sk - np.append(Sk[1:], 0.0)
    ece = np.abs(corr_sum - conf_sum).sum() / N
    return np.asarray(ece, dtype=np.float32)


def kernel(inputs: np.ndarray, targets: np.ndarray) -> np.ndarray:
    r = _run(inputs, targets, trace=False)
    return _combine(r.results)
